# revision 1
# baseline (speedup 1.0000x reference)
"""Trainium2 Bass kernel for nn_DrawImageLayer (draw Gaussian strokes, max over time).

out[b, i, j, 0] = min(1, max_t I[b,t] * exp(-g*(r_i-y[b,t])^2) * exp(-g*(r_j-x[b,t])^2))

Data parallel over 8 NeuronCores: 128 batch rows per core (= SBUF partitions).
Self-contained: hardcodes shapes (B=1024, T=64, SIZE=28) per the problem spec.
"""

import numpy as np

import concourse.bass as bass
import concourse.mybir as mybir
from concourse.bass_utils import run_bass_kernel_spmd

SIZE = 28
T = 64
BC = 128  # batch rows per core
NCORES = 8
P2 = SIZE * SIZE
TI = T * SIZE
G = (SIZE / 2.0) ** 2
SQRT_G = float(np.sqrt(G))
F32 = mybir.dt.float32


def _ap(t, offset, dims):
    """AP over sbuf tensor t: partition dim [row_pitch, 128] + free dims."""
    return bass.AP(t, offset, [[t.shape[1], BC]] + [list(d) for d in dims])


def build(rep: int = 1) -> bass.Bass:
    """Raw-bass program for one core (SPMD across 8).

    DVE does: dy/dx grids (2 TT), intensity fold (1 TT), then per stroke t a
    broadcast-product TT + running-max TT. ACT does the two square+exp chains.
    """
    nc = bass.Bass()
    xin = nc.declare_dram_parameter("xin", [BC, T * 3], F32, isOutput=False)
    r28 = nc.declare_dram_parameter("r28", [BC, SIZE], F32, isOutput=False)
    out = nc.declare_dram_parameter("out", [BC, P2], F32, isOutput=True)

    AO = mybir.AluOpType
    AF = mybir.ActivationFunctionType

    with (
        nc.sbuf_tensor([BC, T * 3], F32) as xs,
        nc.sbuf_tensor([BC, SIZE], F32) as rs,
        nc.sbuf_tensor([BC, TI], F32) as d1,
        nc.sbuf_tensor([BC, TI], F32) as d2,
        nc.sbuf_tensor([BC, TI], F32) as py,
        nc.sbuf_tensor([BC, TI], F32) as pxi,
        nc.sbuf_tensor([BC, P2], F32) as scr,
        nc.sbuf_tensor([BC, P2], F32) as img,
        nc.semaphore("dma_sem") as dma_sem,
        nc.semaphore("va") as va,
        nc.semaphore("av") as av,
        nc.semaphore("vd") as vd,
        nc.Block() as block,
    ):

        @block.sync
        def _(sync):
            for k in range(rep):
                sync.dma_start(out=xs[:, :], in_=xin[:, :]).then_inc(dma_sem, 16)
                sync.dma_start(out=rs[:, :], in_=r28[:, :]).then_inc(dma_sem, 16)
                sync.wait_ge(vd, k + 1)
                sync.dma_start(out=out[:, :], in_=img[:, :]).then_inc(dma_sem, 16)
            sync.wait_ge(dma_sem, rep * 48)

        @block.vector
        def _(vector):
            for k in range(rep):
                vector.wait_ge(dma_sem, 48 * k + 32)
                # dy[b,(t,i)] = r_i - y[b,t]
                nc.vector.tensor_tensor(
                    _ap(d1, 0, [[SIZE, T], [1, SIZE]]),
                    _ap(rs, 0, [[0, T], [1, SIZE]]),
                    _ap(xs, 1, [[3, T], [0, SIZE]]),
                    AO.subtract,
                ).then_inc(va, 1)
                # dx[b,(t,j)] = r_j - x[b,t]
                nc.vector.tensor_tensor(
                    _ap(d2, 0, [[SIZE, T], [1, SIZE]]),
                    _ap(rs, 0, [[0, T], [1, SIZE]]),
                    _ap(xs, 0, [[3, T], [0, SIZE]]),
                    AO.subtract,
                ).then_inc(va, 1)
                vector.wait_ge(av, 2 * k + 2)
                # pxi[b,(t,j)] = exp(-g dx^2) * I[b,t]
                nc.vector.tensor_tensor(
                    _ap(pxi, 0, [[1, TI]]),
                    _ap(d2, 0, [[1, TI]]),
                    _ap(xs, 2, [[3, T], [0, SIZE]]),
                    AO.mult,
                )
                vector.drain()
                for t in range(T):
                    in_py = _ap(py, t * SIZE, [[1, SIZE], [0, SIZE]])
                    in_px = _ap(pxi, t * SIZE, [[0, SIZE], [1, SIZE]])
                    if t == 0:
                        nc.vector.tensor_tensor(
                            _ap(img, 0, [[1, P2]]), in_py, in_px, AO.mult
                        )
                    else:
                        nc.vector.tensor_tensor(
                            _ap(scr, 0, [[1, P2]]), in_py, in_px, AO.mult
                        )
                        vector.drain()
                        nc.vector.tensor_tensor(
                            _ap(img, 0, [[1, P2]]),
                            _ap(img, 0, [[1, P2]]),
                            _ap(scr, 0, [[1, P2]]),
                            AO.max,
                        )
                    vector.drain()
                nc.vector.tensor_scalar_min(
                    _ap(img, 0, [[1, P2]]), _ap(img, 0, [[1, P2]]), 1.0
                ).then_inc(vd, 1)

        @block.scalar
        def _(scalar):
            for k in range(rep):
                scalar.wait_ge(va, 2 * k + 1)
                nc.scalar.activation(
                    _ap(d1, 0, [[1, TI]]), _ap(d1, 0, [[1, TI]]), AF.Square, scale=SQRT_G
                )
                scalar.drain()
                nc.scalar.activation(
                    _ap(py, 0, [[1, TI]]), _ap(d1, 0, [[1, TI]]), AF.Exp, scale=-1.0
                ).then_inc(av, 1)
                scalar.wait_ge(va, 2 * k + 2)
                nc.scalar.activation(
                    _ap(d2, 0, [[1, TI]]), _ap(d2, 0, [[1, TI]]), AF.Square, scale=SQRT_G
                )
                scalar.drain()
                nc.scalar.activation(
                    _ap(d2, 0, [[1, TI]]), _ap(d2, 0, [[1, TI]]), AF.Exp, scale=-1.0
                ).then_inc(av, 1)

    return nc


_GRID = (np.arange(SIZE, dtype=np.float32) / SIZE - 0.5).astype(np.float32)


def make_in_maps(x: np.ndarray) -> list:
    """Shard x (1024, 64, 3) -> per-core input maps."""
    r28 = np.ascontiguousarray(np.broadcast_to(_GRID, (BC, SIZE)))
    maps = []
    for c in range(NCORES):
        xc = np.ascontiguousarray(
            x[c * BC : (c + 1) * BC].reshape(BC, T * 3), dtype=np.float32
        )
        maps.append({"xin": xc, "r28": r28})
    return maps


def kernel(x: np.ndarray) -> np.ndarray:
    """Full-input entry point: x (1024, 64, 3) fp32 -> (1024, 28, 28, 1) fp32."""
    x = np.asarray(x, dtype=np.float32)
    assert x.shape == (1024, T, 3), x.shape
    nc = build(rep=1)
    res = run_bass_kernel_spmd(nc, make_in_maps(x), list(range(NCORES)))
    outs = [res.results[c]["out"].reshape(BC, SIZE, SIZE, 1) for c in range(NCORES)]
    return np.concatenate(outs, axis=0)


# revision 2
# speedup vs baseline: 13.6480x; 13.6480x over previous
"""Trainium2 Bass kernel for nn_DrawImageLayer (draw Gaussian strokes, max over time).

Reference semantics:
  out[b,i,j,0] = min(1, max_t I[b,t] * exp(-g*(r_i - y[b,t])^2) * exp(-g*(r_j - x[b,t])^2))
  r_k = k/28 - 0.5, g = (28/2)^2 = 196, shapes B=1024, T=64, canvas 28x28.

Strategy: pure data parallel — 128 batch rows per NeuronCore (= SBUF
partitions) across 8 cores. Compute in LOG domain so exp commutes with the
max and runs once on the final 784 pixels:
  out = exp( max_t [ (lnI[b,t] - g*dx[b,t,j]^2) - g*dy[b,t,i]^2 ] )
The min(.,1) clamp is dropped: I < 1 strictly => all log values < 0.

On this runtime every engine instruction costs ~25-50us nearly independent of
operand size, so the kernel is built from the fewest, largest ops (~16
instructions total per core):
  DVE : d12 = r - [y|x]            one 3584-elem sub (halves via a concat-AP
                                   trick: offset dim [-1,2] flips channel)
        ex  = lnI - g*dx^2         one 1792-elem sub (lnI broadcast over j)
        cube[(i,j,t)] = ex - g*dy^2  two 25088-elem subs (image halves,
                                   3-free-dim APs with stride-0 broadcasts)
        reduce max over t          two segmented tensor_reduce (t innermost)
  ACT : Square(sqrt(g)*d12), Ln(I), final Exp(784)
"""

from contextlib import ExitStack

import numpy as np

import concourse.bass as bass
import concourse.mybir as mybir
from concourse.bass_utils import run_bass_kernel_spmd

SIZE = 28
T = 64
B = 1024
BC = 128  # batch rows per core
NCORES = 8
P2 = SIZE * SIZE
TI = T * SIZE  # 1792
G = (SIZE / 2.0) ** 2
SQRT_G = float(np.sqrt(G))
F32 = mybir.dt.float32
AO = mybir.AluOpType
AF = mybir.ActivationFunctionType
IH = SIZE // 2  # image rows per half-cube
CUBE = IH * SIZE * T  # 25088
RSOFF = T * 3  # grid columns appended after the (t,c) block
XCOLS = RSOFF + SIZE

_GRID = (np.arange(SIZE, dtype=np.float32) / SIZE - 0.5).astype(np.float32)


def _ap(t, offset, dims):
    """AP over an sbuf tensor: partition dim [row_pitch, 128] + free dims."""
    return bass.AP(t, offset, [[t.shape[1], BC]] + [list(d) for d in dims])


def build(rep: int = 1, drains: bool = False) -> bass.Bass:
    """One-core program, SPMD across 8 cores. rep>1 replicates the body
    (cumulative semaphore thresholds) for wall-clock delta timing."""
    nc = bass.Bass(detect_race_conditions=drains)
    xin = nc.declare_dram_parameter("xin", [BC, XCOLS], F32, isOutput=False)
    out = nc.declare_dram_parameter("out", [BC, P2], F32, isOutput=True)

    with ExitStack() as ctx:
        xs = ctx.enter_context(nc.sbuf_tensor([BC, XCOLS], F32))
        d12 = ctx.enter_context(nc.sbuf_tensor([BC, 2 * TI], F32))
        s12 = ctx.enter_context(nc.sbuf_tensor([BC, 2 * TI], F32))
        lnv = ctx.enter_context(nc.sbuf_tensor([BC, T], F32))
        ex = ctx.enter_context(nc.sbuf_tensor([BC, TI], F32))
        cube = ctx.enter_context(nc.sbuf_tensor([BC, CUBE], F32))
        img = ctx.enter_context(nc.sbuf_tensor([BC, P2], F32))
        dma_sem = ctx.enter_context(nc.semaphore("dma_sem"))
        va = ctx.enter_context(nc.semaphore("va"))  # vector -> scalar
        av = ctx.enter_context(nc.semaphore("av"))  # scalar -> vector
        vd = ctx.enter_context(nc.semaphore("vd"))  # scalar(exp) -> out dma
        block = ctx.enter_context(nc.Block())

        @block.sync
        def _(sync):
            for k in range(rep):
                if k > 0:
                    sync.wait_ge(av, 2 * k)  # prev Square+Ln done => xs consumed
                sync.dma_start(out=xs[:, :], in_=xin[:, :]).then_inc(dma_sem, 16)
                sync.wait_ge(vd, k + 1)
                sync.dma_start(out=out[:, :], in_=img[:, :]).then_inc(dma_sem, 16)
            sync.wait_ge(dma_sem, rep * 32)

        @block.vector
        def _(vector):
            for k in range(rep):
                vector.wait_ge(dma_sem, k * 32 + 16)
                # d12 = r - [y | x]  (y at channel 1, x at channel 0)
                nc.vector.tensor_tensor(
                    _ap(d12, 0, [[1, 2 * TI]]),
                    _ap(xs, RSOFF, [[0, 2], [0, T], [1, SIZE]]),
                    _ap(xs, 1, [[-1, 2], [3, T], [0, SIZE]]),
                    AO.subtract,
                ).then_inc(va, 1)
                vector.wait_ge(av, 2 * k + 2)  # Square + Ln done
                # ex[t*28+j] = lnI[t] - g*dx^2[t*28+j]
                nc.vector.tensor_tensor(
                    _ap(ex, 0, [[1, TI]]),
                    _ap(lnv, 0, [[1, T], [0, SIZE]]),
                    _ap(s12, TI, [[1, TI]]),
                    AO.subtract,
                )
                if drains:
                    vector.drain()
                for h in range(2):
                    if drains and h > 0:
                        vector.drain()
                    # cube[(i,j,t)] = ex[t*28+j] - g*dy^2[t*28+i], i in half h
                    nc.vector.tensor_tensor(
                        _ap(cube, 0, [[1, CUBE]]),
                        _ap(ex, 0, [[0, IH], [1, SIZE], [SIZE, T]]),
                        _ap(s12, h * IH, [[1, IH], [0, SIZE], [SIZE, T]]),
                        AO.subtract,
                    )
                    if drains:
                        vector.drain()
                    red = nc.vector.tensor_reduce(
                        _ap(img, h * IH * SIZE, [[1, IH * SIZE]]),
                        _ap(cube, 0, [[SIZE * T, IH], [T, SIZE], [1, T]]),
                        mybir.AxisListType.X,
                        AO.max,
                    )
                red.then_inc(va, 1)

        @block.scalar
        def _(scalar):
            for k in range(rep):
                scalar.wait_ge(va, 2 * k + 1)
                nc.scalar.activation(
                    _ap(s12, 0, [[1, 2 * TI]]),
                    _ap(d12, 0, [[1, 2 * TI]]),
                    AF.Square,
                    scale=SQRT_G,
                ).then_inc(av, 1)
                nc.scalar.activation(
                    _ap(lnv, 0, [[1, T]]),
                    _ap(xs, 2, [[3, T]]),
                    AF.Ln,
                ).then_inc(av, 1)
                scalar.wait_ge(va, 2 * k + 2)  # log-image complete
                nc.scalar.activation(
                    _ap(img, 0, [[1, P2]]),
                    _ap(img, 0, [[1, P2]]),
                    AF.Exp,
                ).then_inc(vd, 1)

    return nc


def make_in_maps(x: np.ndarray) -> list:
    """Shard x (1024, 64, 3) -> per-core maps; grid constant appended."""
    maps = []
    for c in range(NCORES):
        xc = x[c * BC : (c + 1) * BC].reshape(BC, T * 3).astype(np.float32)
        xc = np.concatenate([xc, np.broadcast_to(_GRID, (BC, SIZE))], axis=1)
        maps.append({"xin": np.ascontiguousarray(xc)})
    return maps


def kernel(x: np.ndarray) -> np.ndarray:
    """Full inputs in, full output out: (1024, 64, 3) f32 -> (1024, 28, 28, 1) f32."""
    x = np.asarray(x, dtype=np.float32)
    assert x.shape == (B, T, 3), x.shape
    nc = build(rep=1)
    res = run_bass_kernel_spmd(nc, make_in_maps(x), list(range(NCORES)))
    outs = [res.results[c]["out"].reshape(BC, SIZE, SIZE, 1) for c in range(NCORES)]
    return np.concatenate(outs, axis=0)
